# revision 37
# baseline (speedup 1.0000x reference)
"""Trainium2 Bass kernel for the CSMHP (clustered self-exciting Hawkes process)
negative log-likelihood, distributed over 8 NeuronCores.

Math
----
The reference builds the full (C, N, N) pairwise decay tensor and row-reduces
it with logsumexp.  The excitation

    E[c, i] = sum_{j<i} exp(-beta_c * (t_i - t_j))

obeys the first-order recurrence  E_i = d_i * (E_{i-1} + 1)  with
d_i = exp(-beta_c * (t_i - t_{i-1})), which maps exactly onto the DVE
`tensor_tensor_scan` instruction: state = (d *mult* state) *add* d.
That turns the O(N^2 C) pairwise tensor into O(N C) work.

Sharding
--------
Events are split into 8 contiguous blocks of 512 (the N axis of the pairwise
tensor, as the hint suggests).  Each core:
  * computes its scan-initial state A'[c] = E[c, first_own_event - 1] directly
    from the (padded, uniform-shape) list of prior events — a dense
    exp+reduce over at most 3584 values, so no cross-core recurrence and no
    collectives are needed;
  * runs the scan over its 512 events for all 8 clusters at once;
  * reduces its partial log-likelihood sum, its partial probability column
    sum, and (core 7) the excitation at the very last event, which is exactly
    the data the analytic integral term needs.
The host sums the 8 partial scalars (the "all-reduce" of the hint).
"""

import numpy as np

import concourse.bass as bass
import concourse.tile as tile
from concourse import mybir
from concourse.bass_utils import run_bass_kernel_spmd
from concourse.vector_clock import ScopedClock

F32 = mybir.dt.float32
ALU = mybir.AluOpType
ACT = mybir.ActivationFunctionType

N = 4096
C = 8
NCORES = 8
CHUNK = N // NCORES          # 512 events per core
PRIOR_PAD = 3584             # max prior events (core 7: 512*7-1=3583), padded
PCOLS = PRIOR_PAD // 128     # 28
T_WINDOW = 100.0
BIG = 1.0e9                  # pad offset: exp(-beta*BIG) == 0 in fp32

_NC_CACHE = None


def _build_nc(with_clears: bool = True):
    """Raw-Bass (no TileContext) SPMD program run on every core.

    Hand-placed semaphores; no entry/exit barriers, no drains.  Every
    compute instruction carries at most one embedded semaphore wait (the
    TPB encoding limit); where a second wait is needed it is emitted as a
    standalone wait_ge on the engine queue.

    Engine streams:
      SP    : inA DMA, inB DMA (HWDGE), then the single output DMA
      DVE   : pre-scan prep, the excitation scan, intensity products, staging
      ACT   : the two exp's, PSUM bounce, fused Ln+sum
      PE    : history matmul, intensity matmul
    """
    nc = bass.Bass("TRN2", target_bir_lowering=False, debug=False)

    # inA columns: [0:512] t_own | [512:1024] t_prev | [1024:1536] pT
    #              [1536:1540] scal (beta, alpha, mu, gamma)
    ina_d = nc.dram_tensor("inA", [C, 3 * CHUNK + 4], F32, kind="ExternalInput")
    # inB columns: [0:224] prior_rep | [224:225] tref | [225:449] b128_rep
    #              [449:457] ones
    CP = C * PCOLS
    inb_d = nc.dram_tensor("inB", [128, 2 * CP + 1 + C], F32, kind="ExternalInput")
    # out columns: 0 = probability column sums, 1 = last-event excitation,
    # 2 = ll partial (all rows identical)
    out_d = nc.dram_tensor("out", [C, 3], F32, kind="ExternalOutput")

    from contextlib import ExitStack

    ctx = ExitStack()
    sb = lambda name, shape: ctx.enter_context(nc.sbuf_tensor(name, shape, F32))
    psum = lambda name, shape: ctx.enter_context(
        nc.psum_tensor(name, shape, F32)
    )
    sem = lambda name: ctx.enter_context(nc.semaphore(name))
    with ctx:
        ina = sb("ina", [C, 3 * CHUNK + 4])
        inb = sb("inb", [128, 2 * CP + 1 + C])
        wbig = sb("wbig", [128, CP])
        ebig = sb("ebig", [128, C, PCOLS])
        r_part = sb("r_part", [128, C])
        a_init_sb = sb("a_init_sb", [C, 1])
        dt = sb("dt", [C, CHUNK])
        negb = sb("negb", [C, 1])
        dec = sb("dec", [C, CHUNK])
        baset = sb("baset", [C, CHUNK])
        base = sb("base", [C, CHUNK])
        exc = sb("exc", [C, CHUNK])
        lamb = sb("lamb", [C, CHUNK])
        pl = sb("pl", [C, CHUNK])
        logi = sb("logi", [C, CHUNK])
        ll = sb("ll", [C, 1])
        out_stage = sb("out_stage", [C, 3])
        a_init = psum("a_init", [C, 1])
        inten = psum("inten", [C, CHUNK])
        s_in = sem("s_in")
        s_dve = sem("s_dve")
        s_act = sem("s_act")
        s_pe = sem("s_pe")
        s_stage = sem("s_stage")
        s_out = sem("s_out")
        s_v = sem("s_v")  # DVE same-engine RAW handshakes (pipeline not interlocked)
        block = ctx.enter_context(nc.Block())

        ina_ap = ina.ap()
        t_own = ina_ap[:, 0:CHUNK]
        t_prev = ina_ap[:, CHUNK : 2 * CHUNK]
        pt = ina_ap[:, 2 * CHUNK : 3 * CHUNK]
        scal = ina_ap[:, 3 * CHUNK : 3 * CHUNK + 4]
        inb_ap = inb.ap()
        prior_rep = inb_ap[:, 0:CP]
        tref = inb_ap[:, CP : CP + 1]
        b128_rep = inb_ap[:, CP + 1 : 2 * CP + 1]
        ones_in = inb_ap[:, 2 * CP + 1 : 2 * CP + 1 + C]

        beta_col = scal[:, 0:1]
        alpha_col = scal[:, 1:2]
        mu_col = scal[:, 2:3]
        gamma_col = scal[:, 3:4]

        @block.sync
        def _(sync):
            sync.dma_start(out=ina.ap(), in_=ina_d.ap()).then_inc(s_in, 16)
            sync.dma_start(out=inb.ap(), in_=inb_d.ap()).then_inc(s_in, 16)
            # out_stage complete after 3 staged DVE writes
            sync.wait_ge(s_stage, 3)
            sync.dma_start(out=out_d.ap(), in_=out_stage.ap()).then_inc(
                s_out, 16
            )
            sync.wait_ge(s_out, 16)
            # reset for the next execution of the same loaded NEFF: at this
            # point every other stream has retired (s_out implies s_stage
            # implies DVE done implies ACT done implies PE done, and the
            # input DMAs certainly completed).  The sim's race checker
            # doesn't track that transitivity, so sim builds skip the
            # clears (the sim only executes once anyway).
            if with_clears:
                sync.wait_ge(s_in, 32)
                for s in (s_in, s_dve, s_act, s_pe, s_stage, s_out, s_v):
                    sync.sem_clear(s)

        @block.vector
        def _(vector):
            vector.wait_ge(s_in, 32)
            # pre-scan prep; wbig first: it gates ACT
            nc.vector.scalar_tensor_tensor(
                out=wbig.ap(), in0=prior_rep, scalar=tref, in1=b128_rep,
                op0=ALU.subtract, op1=ALU.mult,
            ).then_inc(s_dve, 1)                                   # s_dve 1
            nc.vector.tensor_scalar_mul(negb.ap(), beta_col, -1.0).then_inc(
                s_dve, 1
            )                                                      # s_dve 2
            nc.vector.tensor_sub(dt.ap(), t_own, t_prev).then_inc(
                s_dve, 1
            )                                                      # s_dve 3
            nc.vector.reduce_sum(
                out_stage.ap()[:, 0:1], pt, axis=mybir.AxisListType.X
            ).then_inc(s_stage, 1)                                 # s_stage 1
            nc.vector.tensor_scalar(
                out=baset.ap(), in0=t_own, scalar1=1.0 / T_WINDOW,
                scalar2=gamma_col, op0=ALU.mult, op1=ALU.mult,
            ).then_inc(s_v, 1)                                     # s_v 1
            vector.wait_ge(s_v, 1)
            nc.vector.tensor_scalar(
                out=base.ap(), in0=baset.ap(), scalar1=mu_col, scalar2=None,
                op0=ALU.add,
            )
            # r_part needs ebig (ACT #1)
            vector.wait_ge(s_act, 1)
            nc.vector.reduce_sum(
                r_part.ap(), ebig.ap(), axis=mybir.AxisListType.X
            ).then_inc(s_dve, 1)                                   # s_dve 4
            # the scan needs dec (ACT #2) and a_init_sb (ACT #3)
            vector.wait_ge(s_act, 3)
            nc.vector.tensor_tensor_scan(
                exc.ap(), dec.ap(), dec.ap(), initial=a_init_sb.ap(),
                op0=ALU.mult, op1=ALU.add,
            ).then_inc(s_v, 1)                                     # s_v 2
            vector.wait_ge(s_v, 2)
            nc.vector.scalar_tensor_tensor(
                out=lamb.ap(), in0=exc.ap(), scalar=alpha_col, in1=base.ap(),
                op0=ALU.mult, op1=ALU.add,
            ).then_inc(s_v, 1)                                     # s_v 3
            vector.wait_ge(s_v, 3)
            nc.vector.tensor_mul(pl.ap(), lamb.ap(), pt).then_inc(
                s_dve, 1
            )                                                      # s_dve 5
            nc.vector.tensor_copy(
                out_stage.ap()[:, 1:2], exc.ap()[:, CHUNK - 1 : CHUNK]
            ).then_inc(s_stage, 1)                                 # s_stage 2
            # ll needs the fused Ln accum (ACT #4)
            vector.wait_ge(s_act, 4)
            nc.vector.tensor_copy(out_stage.ap()[:, 2:3], ll.ap()).then_inc(
                s_stage, 1
            )                                                      # s_stage 3

        @block.scalar
        def _(scalar):
            # ebig needs wbig (DVE #1)
            scalar.wait_ge(s_dve, 1)
            nc.scalar.activation(
                ebig.ap(), wbig.ap().rearrange("p (c f) -> p c f", c=C),
                ACT.Exp,
            ).then_inc(s_act, 1)                                   # s_act 1
            # dec needs dt + negb (DVE #2,#3)
            scalar.wait_ge(s_dve, 3)
            nc.scalar.activation(
                dec.ap(), dt.ap(), ACT.Exp, scale=negb.ap()
            ).then_inc(s_act, 1)                                   # s_act 2
            # PSUM bounce needs the history matmul (PE #1)
            scalar.wait_ge(s_pe, 1)
            nc.scalar.copy(a_init_sb.ap(), a_init.ap()).then_inc(
                s_act, 1
            )                                                      # s_act 3
            # fused Ln + sum needs the intensity matmul (PE #2)
            scalar.wait_ge(s_pe, 2)
            nc.scalar.activation(
                logi.ap(), inten.ap(), ACT.Ln, accum_out=ll.ap()
            ).then_inc(s_act, 1)                                   # s_act 4

        @block.tensor
        def _(tensor):
            # ones live in SBUF straight from the inB DMA; lhsT also wants
            # r_part (DVE #3)
            tensor.wait_ge(s_dve, 4)
            nc.tensor.matmul(
                a_init.ap(), r_part.ap(), ones_in[:, 0:1],
                start=True, stop=True,
            ).then_inc(s_pe, 1)                                    # s_pe 1
            tensor.wait_ge(s_dve, 5)
            nc.tensor.matmul(
                inten.ap(), ones_in[0:C, :], pl.ap(), start=True, stop=True
            ).then_inc(s_pe, 1)                                    # s_pe 2

    return nc


def get_nc(with_clears: bool = True):
    global _NC_CACHE
    if _NC_CACHE is None:
        _NC_CACHE = _build_nc(with_clears)
    return _NC_CACHE


def make_in_maps(probability, event_times, mu, gamma, alpha_kernel, beta_kernel):
    t = np.ascontiguousarray(np.asarray(event_times, dtype=np.float32))
    p = np.ascontiguousarray(np.asarray(probability, dtype=np.float32))
    beta = np.asarray(beta_kernel, dtype=np.float32)
    alpha = np.asarray(alpha_kernel, dtype=np.float32)
    mu_ = np.asarray(mu, dtype=np.float32)
    gamma_ = np.asarray(gamma, dtype=np.float32)

    scal = np.stack([beta, alpha, mu_, gamma_], axis=1)
    b128 = np.broadcast_to(beta, (128, C))

    in_maps = []
    for k in range(NCORES):
        s = k * CHUNK
        t_own = np.broadcast_to(t[s : s + CHUNK], (C, CHUNK))
        tp = np.empty(CHUNK, np.float32)
        if k == 0:
            tp[0] = t[0] - BIG  # forces d_0 = 0: no events precede event 0
            tp[1:] = t[: CHUNK - 1]
        else:
            tp[:] = t[s - 1 : s + CHUNK - 1]
        t_prev = np.broadcast_to(tp, (C, CHUNK))
        pt = p[s : s + CHUNK, :].T

        npri = max(s - 1, 0)
        pri = np.full(PRIOR_PAD, -BIG, np.float32)
        pri[:npri] = t[:npri]
        prior_pm = pri.reshape(PCOLS, 128).T
        tref_val = t[s - 1] if k > 0 else t[0]
        tref = np.full((128, 1), tref_val, np.float32)

        ina = np.ascontiguousarray(
            np.concatenate([t_own, t_prev, pt, scal], axis=1, dtype=np.float32)
        )
        ones_c = np.ones((128, C), np.float32)
        prior_rep = np.tile(prior_pm, (1, C))                       # (128, 224)
        b128_rep = np.broadcast_to(
            np.repeat(beta, PCOLS)[None, :], (128, C * PCOLS)
        )
        inb = np.ascontiguousarray(
            np.concatenate(
                [prior_rep, tref, b128_rep, ones_c], axis=1, dtype=np.float32
            )
        )
        in_maps.append({"inA": ina, "inB": inb})
    return in_maps


def combine_outputs(results, event_times, mu, gamma, alpha_kernel, beta_kernel):
    """Host-side reduction of the per-core partial scalars (float64)."""
    t = np.asarray(event_times, dtype=np.float32)
    beta = np.asarray(beta_kernel, dtype=np.float64)
    alpha = np.asarray(alpha_kernel, dtype=np.float64)
    mu_ = np.asarray(mu, dtype=np.float64)
    gamma_ = np.asarray(gamma, dtype=np.float64)

    ll_sum = sum(float(r["out"][0, 2]) for r in results)
    psum = np.zeros(C, np.float64)
    for r in results:
        psum += r["out"][:, 0].astype(np.float64)
    elast = results[NCORES - 1]["out"][:, 1].astype(np.float64)

    ab = alpha / beta
    exp_term = ab * ((N - 1) - elast)
    t_diff = float(t[-1]) - float(t[0])
    t_sq_diff = float(t[-1]) ** 2 - float(t[0]) ** 2
    base_terms = t_diff * mu_ + t_sq_diff * gamma_ / (2.0 * T_WINDOW)
    integral_part = float(psum @ (exp_term + base_terms)) / N
    return np.float32(-(ll_sum - integral_part))


def kernel(probability, event_times, mu, gamma, alpha_kernel, beta_kernel):
    nc = get_nc()
    in_maps = make_in_maps(
        probability, event_times, mu, gamma, alpha_kernel, beta_kernel
    )
    res = run_bass_kernel_spmd(nc, in_maps, core_ids=list(range(NCORES))).results
    return combine_outputs(
        res, event_times, mu, gamma, alpha_kernel, beta_kernel
    )


# revision 38
# speedup vs baseline: 1.2388x; 1.2388x over previous
"""Trainium2 Bass kernel for the CSMHP (clustered self-exciting Hawkes process)
negative log-likelihood, distributed over 8 NeuronCores.

Math
----
The reference builds the full (C, N, N) pairwise decay tensor and row-reduces
it with logsumexp.  The excitation

    E[c, i] = sum_{j<i} exp(-beta_c * (t_i - t_j))

obeys the first-order recurrence  E_i = d_i * (E_{i-1} + 1)  with
d_i = exp(-beta_c * (t_i - t_{i-1})), which maps exactly onto the DVE
`tensor_tensor_scan` instruction: state = (d *mult* state) *add* d.
That turns the O(N^2 C) pairwise tensor into O(N C) work.

Sharding
--------
Events are split into 8 contiguous blocks of 512 (the N axis of the pairwise
tensor, as the hint suggests).  Each core:
  * computes its scan-initial state A'[c] = E[c, first_own_event - 1] directly
    from the (padded, uniform-shape) list of prior events — a dense
    exp+reduce over at most 3584 values, so no cross-core recurrence and no
    collectives are needed;
  * runs the scan over its 512 events for all 8 clusters at once;
  * reduces its partial log-likelihood sum, its partial probability column
    sum, and (core 7) the excitation at the very last event, which is exactly
    the data the analytic integral term needs.
The host sums the 8 partial scalars (the "all-reduce" of the hint).
"""

import numpy as np

import concourse.bass as bass
import concourse.tile as tile
from concourse import mybir
from concourse.bass_utils import run_bass_kernel_spmd
from concourse.vector_clock import ScopedClock

F32 = mybir.dt.float32
ALU = mybir.AluOpType
ACT = mybir.ActivationFunctionType

N = 4096
C = 8
NCORES = 8
CHUNK = N // NCORES          # 512 events per core
PRIOR_PAD = 3584             # max prior events (core 7: 512*7-1=3583), padded
PCOLS = PRIOR_PAD // 128     # 28
T_WINDOW = 100.0
BIG = 1.0e9                  # pad offset: exp(-beta*BIG) == 0 in fp32

_NC_CACHE = None


def _build_nc(with_clears: bool = True):
    """Raw-Bass (no TileContext) SPMD program run on every core.

    Hand-placed semaphores; no entry/exit barriers, no drains.  Every
    compute instruction carries at most one embedded semaphore wait (the
    TPB encoding limit); where a second wait is needed it is emitted as a
    standalone wait_ge on the engine queue.

    Engine streams:
      SP    : inA DMA, inB DMA (HWDGE), then the single output DMA
      DVE   : pre-scan prep, the excitation scan, intensity products, staging
      ACT   : the two exp's, PSUM bounce, fused Ln+sum
      PE    : history matmul, intensity matmul
    """
    nc = bass.Bass("TRN2", target_bir_lowering=False, debug=False)

    # inA columns: [0:512] t_own | [512:1024] t_prev | [1024:1536] pT
    #              [1536:1540] scal (beta, alpha, mu, gamma)
    ina_d = nc.dram_tensor("inA", [C, 3 * CHUNK + 4], F32, kind="ExternalInput")
    # inB columns: [0:224] prior_rep | [224:225] tref | [225:449] b128_rep
    #              [449:457] ones | [457:458] zeros (explicit activation bias)
    CP = C * PCOLS
    inb_d = nc.dram_tensor("inB", [128, 2 * CP + 2 + C], F32, kind="ExternalInput")
    # out columns: 0 = probability column sums, 1 = last-event excitation,
    # 2 = ll partial (all rows identical)
    out_d = nc.dram_tensor("out", [C, 3], F32, kind="ExternalOutput")

    from contextlib import ExitStack

    ctx = ExitStack()
    sb = lambda name, shape: ctx.enter_context(nc.sbuf_tensor(name, shape, F32))
    psum = lambda name, shape: ctx.enter_context(
        nc.psum_tensor(name, shape, F32)
    )
    sem = lambda name: ctx.enter_context(nc.semaphore(name))
    with ctx:
        ina = sb("ina", [C, 3 * CHUNK + 4])
        inb = sb("inb", [128, 2 * CP + 2 + C])
        wbig = sb("wbig", [128, CP])
        ebig = sb("ebig", [128, C, PCOLS])
        r_part = sb("r_part", [128, C])
        a_init_sb = sb("a_init_sb", [C, 1])
        dt = sb("dt", [C, CHUNK])
        negb = sb("negb", [C, 1])
        dec = sb("dec", [C, CHUNK])
        baset = sb("baset", [C, CHUNK])
        base = sb("base", [C, CHUNK])
        exc = sb("exc", [C, CHUNK])
        lamb = sb("lamb", [C, CHUNK])
        pl = sb("pl", [C, CHUNK])
        logi = sb("logi", [C, CHUNK])
        ll = sb("ll", [C, 1])
        out_stage = sb("out_stage", [C, 3])
        a_init = psum("a_init", [C, 1])
        inten = psum("inten", [C, CHUNK])
        s_in = sem("s_in")
        s_dve = sem("s_dve")
        s_act = sem("s_act")
        s_pe = sem("s_pe")
        s_stage = sem("s_stage")
        s_out = sem("s_out")
        s_v = sem("s_v")  # DVE same-engine RAW handshakes (pipeline not interlocked)
        block = ctx.enter_context(nc.Block(no_gpsimd_drain=True))

        ina_ap = ina.ap()
        t_own = ina_ap[:, 0:CHUNK]
        t_prev = ina_ap[:, CHUNK : 2 * CHUNK]
        pt = ina_ap[:, 2 * CHUNK : 3 * CHUNK]
        scal = ina_ap[:, 3 * CHUNK : 3 * CHUNK + 4]
        inb_ap = inb.ap()
        prior_rep = inb_ap[:, 0:CP]
        tref = inb_ap[:, CP : CP + 1]
        b128_rep = inb_ap[:, CP + 1 : 2 * CP + 1]
        ones_in = inb_ap[:, 2 * CP + 1 : 2 * CP + 1 + C]
        zeros128 = inb_ap[:, 2 * CP + 1 + C : 2 * CP + 2 + C]

        beta_col = scal[:, 0:1]
        alpha_col = scal[:, 1:2]
        mu_col = scal[:, 2:3]
        gamma_col = scal[:, 3:4]

        @block.sync
        def _(sync):
            sync.dma_start(out=ina.ap(), in_=ina_d.ap()).then_inc(s_in, 16)
            sync.dma_start(out=inb.ap(), in_=inb_d.ap()).then_inc(s_in, 16)
            # out_stage complete after 3 staged DVE writes
            sync.wait_ge(s_stage, 3)
            sync.dma_start(out=out_d.ap(), in_=out_stage.ap()).then_inc(
                s_out, 16
            )
            sync.wait_ge(s_out, 16)
            # reset for the next execution of the same loaded NEFF: at this
            # point every other stream has retired (s_out implies s_stage
            # implies DVE done implies ACT done implies PE done, and the
            # input DMAs certainly completed).  The sim's race checker
            # doesn't track that transitivity, so sim builds skip the
            # clears (the sim only executes once anyway).
            if with_clears:
                sync.wait_ge(s_in, 32)
                for s in (s_in, s_dve, s_act, s_pe, s_stage, s_out, s_v):
                    sync.sem_clear(s)

        @block.vector
        def _(vector):
            vector.wait_ge(s_in, 32)
            # pre-scan prep; wbig first: it gates ACT
            nc.vector.scalar_tensor_tensor(
                out=wbig.ap(), in0=prior_rep, scalar=tref, in1=b128_rep,
                op0=ALU.subtract, op1=ALU.mult,
            ).then_inc(s_dve, 1)                                   # s_dve 1
            nc.vector.tensor_scalar_mul(negb.ap(), beta_col, -1.0).then_inc(
                s_dve, 1
            )                                                      # s_dve 2
            nc.vector.tensor_sub(dt.ap(), t_own, t_prev).then_inc(
                s_dve, 1
            )                                                      # s_dve 3
            nc.vector.reduce_sum(
                out_stage.ap()[:, 0:1], pt, axis=mybir.AxisListType.X
            ).then_inc(s_stage, 1)                                 # s_stage 1
            nc.vector.tensor_scalar(
                out=baset.ap(), in0=t_own, scalar1=1.0 / T_WINDOW,
                scalar2=gamma_col, op0=ALU.mult, op1=ALU.mult,
            ).then_inc(s_v, 1)                                     # s_v 1
            vector.wait_ge(s_v, 1)
            nc.vector.tensor_scalar(
                out=base.ap(), in0=baset.ap(), scalar1=mu_col, scalar2=None,
                op0=ALU.add,
            )
            # r_part needs ebig (ACT #1)
            vector.wait_ge(s_act, 1)
            nc.vector.reduce_sum(
                r_part.ap(), ebig.ap(), axis=mybir.AxisListType.X
            ).then_inc(s_dve, 1)                                   # s_dve 4
            # the scan needs dec (ACT #2) and a_init_sb (ACT #3)
            vector.wait_ge(s_act, 3)
            nc.vector.tensor_tensor_scan(
                exc.ap(), dec.ap(), dec.ap(), initial=a_init_sb.ap(),
                op0=ALU.mult, op1=ALU.add,
            ).then_inc(s_v, 1)                                     # s_v 2
            vector.wait_ge(s_v, 2)
            nc.vector.scalar_tensor_tensor(
                out=lamb.ap(), in0=exc.ap(), scalar=alpha_col, in1=base.ap(),
                op0=ALU.mult, op1=ALU.add,
            ).then_inc(s_v, 1)                                     # s_v 3
            vector.wait_ge(s_v, 3)
            nc.vector.tensor_mul(pl.ap(), lamb.ap(), pt).then_inc(
                s_dve, 1
            )                                                      # s_dve 5
            nc.vector.tensor_copy(
                out_stage.ap()[:, 1:2], exc.ap()[:, CHUNK - 1 : CHUNK]
            ).then_inc(s_stage, 1)                                 # s_stage 2
            # ll needs the fused Ln accum (ACT #4)
            vector.wait_ge(s_act, 4)
            nc.vector.tensor_copy(out_stage.ap()[:, 2:3], ll.ap()).then_inc(
                s_stage, 1
            )                                                      # s_stage 3

        @block.scalar
        def _(scalar):
            # ebig needs wbig (DVE #1)
            scalar.wait_ge(s_dve, 1)
            nc.scalar.activation(
                ebig.ap(), wbig.ap().rearrange("p (c f) -> p c f", c=C),
                ACT.Exp, bias=zeros128,
            ).then_inc(s_act, 1)                                   # s_act 1
            # dec needs dt + negb (DVE #2,#3)
            scalar.wait_ge(s_dve, 3)
            nc.scalar.activation(
                dec.ap(), dt.ap(), ACT.Exp, bias=zeros128[0:C, :],
                scale=negb.ap(),
            ).then_inc(s_act, 1)                                   # s_act 2
            # PSUM bounce needs the history matmul (PE #1)
            scalar.wait_ge(s_pe, 1)
            nc.scalar.copy(a_init_sb.ap(), a_init.ap()).then_inc(
                s_act, 1
            )                                                      # s_act 3
            # fused Ln + sum needs the intensity matmul (PE #2)
            scalar.wait_ge(s_pe, 2)
            nc.scalar.activation(
                logi.ap(), inten.ap(), ACT.Ln, bias=zeros128[0:C, :],
                accum_out=ll.ap(),
            ).then_inc(s_act, 1)                                   # s_act 4

        @block.tensor
        def _(tensor):
            # ones live in SBUF straight from the inB DMA; lhsT also wants
            # r_part (DVE #3)
            tensor.wait_ge(s_dve, 4)
            nc.tensor.matmul(
                a_init.ap(), r_part.ap(), ones_in[:, 0:1],
                start=True, stop=True,
            ).then_inc(s_pe, 1)                                    # s_pe 1
            tensor.wait_ge(s_dve, 5)
            nc.tensor.matmul(
                inten.ap(), ones_in[0:C, :], pl.ap(), start=True, stop=True
            ).then_inc(s_pe, 1)                                    # s_pe 2

    _strip_entry_scaffolding(nc)
    return nc


def _strip_entry_scaffolding(nc):
    """Remove the const-AP Pool memsets and the Bass.__init__ all-engine
    barrier from the entry block.  No instruction in this program reads the
    const APs (activation biases are explicit inB columns), and the
    inter-execution fence the barrier provides is already guaranteed by the
    runtime (execution N+1 starts only after N's queues fully retire).
    Pool's dge_drain in that barrier otherwise delays the input DMAs by
    ~3.5us."""
    main = nc.m.functions[0].blocks[0]
    drop_types = ("InstMemset", "InstDrain", "InstEventSemaphore")
    kept = [
        inst
        for inst in main.instructions
        if type(inst).__name__ not in drop_types
    ]
    main.instructions[:] = kept


def get_nc(with_clears: bool = True):
    global _NC_CACHE
    if _NC_CACHE is None:
        _NC_CACHE = _build_nc(with_clears)
    return _NC_CACHE


def make_in_maps(probability, event_times, mu, gamma, alpha_kernel, beta_kernel):
    t = np.ascontiguousarray(np.asarray(event_times, dtype=np.float32))
    p = np.ascontiguousarray(np.asarray(probability, dtype=np.float32))
    beta = np.asarray(beta_kernel, dtype=np.float32)
    alpha = np.asarray(alpha_kernel, dtype=np.float32)
    mu_ = np.asarray(mu, dtype=np.float32)
    gamma_ = np.asarray(gamma, dtype=np.float32)

    scal = np.stack([beta, alpha, mu_, gamma_], axis=1)
    b128 = np.broadcast_to(beta, (128, C))

    in_maps = []
    for k in range(NCORES):
        s = k * CHUNK
        t_own = np.broadcast_to(t[s : s + CHUNK], (C, CHUNK))
        tp = np.empty(CHUNK, np.float32)
        if k == 0:
            tp[0] = t[0] - BIG  # forces d_0 = 0: no events precede event 0
            tp[1:] = t[: CHUNK - 1]
        else:
            tp[:] = t[s - 1 : s + CHUNK - 1]
        t_prev = np.broadcast_to(tp, (C, CHUNK))
        pt = p[s : s + CHUNK, :].T

        npri = max(s - 1, 0)
        pri = np.full(PRIOR_PAD, -BIG, np.float32)
        pri[:npri] = t[:npri]
        prior_pm = pri.reshape(PCOLS, 128).T
        tref_val = t[s - 1] if k > 0 else t[0]
        tref = np.full((128, 1), tref_val, np.float32)

        ina = np.ascontiguousarray(
            np.concatenate([t_own, t_prev, pt, scal], axis=1, dtype=np.float32)
        )
        ones_c = np.ones((128, C), np.float32)
        prior_rep = np.tile(prior_pm, (1, C))                       # (128, 224)
        b128_rep = np.broadcast_to(
            np.repeat(beta, PCOLS)[None, :], (128, C * PCOLS)
        )
        zeros_c = np.zeros((128, 1), np.float32)
        inb = np.ascontiguousarray(
            np.concatenate(
                [prior_rep, tref, b128_rep, ones_c, zeros_c],
                axis=1, dtype=np.float32,
            )
        )
        in_maps.append({"inA": ina, "inB": inb})
    return in_maps


def combine_outputs(results, event_times, mu, gamma, alpha_kernel, beta_kernel):
    """Host-side reduction of the per-core partial scalars (float64)."""
    t = np.asarray(event_times, dtype=np.float32)
    beta = np.asarray(beta_kernel, dtype=np.float64)
    alpha = np.asarray(alpha_kernel, dtype=np.float64)
    mu_ = np.asarray(mu, dtype=np.float64)
    gamma_ = np.asarray(gamma, dtype=np.float64)

    ll_sum = sum(float(r["out"][0, 2]) for r in results)
    psum = np.zeros(C, np.float64)
    for r in results:
        psum += r["out"][:, 0].astype(np.float64)
    elast = results[NCORES - 1]["out"][:, 1].astype(np.float64)

    ab = alpha / beta
    exp_term = ab * ((N - 1) - elast)
    t_diff = float(t[-1]) - float(t[0])
    t_sq_diff = float(t[-1]) ** 2 - float(t[0]) ** 2
    base_terms = t_diff * mu_ + t_sq_diff * gamma_ / (2.0 * T_WINDOW)
    integral_part = float(psum @ (exp_term + base_terms)) / N
    return np.float32(-(ll_sum - integral_part))


def kernel(probability, event_times, mu, gamma, alpha_kernel, beta_kernel):
    nc = get_nc()
    in_maps = make_in_maps(
        probability, event_times, mu, gamma, alpha_kernel, beta_kernel
    )
    res = run_bass_kernel_spmd(nc, in_maps, core_ids=list(range(NCORES))).results
    return combine_outputs(
        res, event_times, mu, gamma, alpha_kernel, beta_kernel
    )
